# revision 44
# baseline (speedup 1.0000x reference)
"""GCN (3x GCNConv + BN + ReLU, mean-pool, 2-layer MLP) on 8 Trainium2 cores.

Strategy (dst-sharded message passing, V2):
  - Nodes are dst-sharded: core c owns nodes [c*SH, (c+1)*SH).
  - Symmetric norm factorizes: out[i] = dinv[i] * sum_e dinv[src]*h'[src]
    so rows are scaled once (hhat = dinv * (h @ W)); dinv is host-precomputed.
  - hhat is exchanged in 4 quarter-window AllGathers (window p = quarter p of
    every core's shard, < 32768 rows for int16 gather indices) so gathers for
    pass p overlap the collective for pass p+1.
  - Per layer: dma_gather pulls 256B message rows from the window in HBM for
    the edges whose dst is local; a one-hot matmul segment-sums them in PSUM.
    One-hot S tiles are HOST-PRECOMPUTED fp8 and streamed from HBM (keeps the
    Vector engine and the GpSimd SWDGE descriptor generator from fighting
    over their shared SBUF port).
  - Edges bucketed by (pass window, dst block of 128), tiles padded to 128
    with a structure common to all 8 cores (single SPMD NEFF); pad slots in a
    call's final bucket carry idx=-1 so the SWDGE trims their descriptors.
  - Mean-pool via host-precomputed fp8 graph-onehot matmuls, AllReduce, then
    the classifier MLP runs (redundantly) on every core.
"""

import math
from contextlib import ExitStack

import numpy as np

NCORES = 8
NUM_GRAPHS = 1000  # G for the graded problem (not derivable from input shapes)
EPS = 1e-5

BLK = 128          # dst nodes per block (= one-hot matmul output partitions)
GSIZE = 6          # dst blocks whose PSUM accumulators are live at once
GTILES_CAP = 8     # max tiles per dma_gather call (64-desc/engine packet ceiling)
FP8_ONE = 0x38     # float8_e4m3 encoding of 1.0

DMA_SCRATCH = 16384    # SWDGE descriptor carveout (bytes per partition)
NQUEUES = 4            # SWDGE queues to round-robin gather calls over
SINGLE_PACKET = True   # dma_gather packetization mode
PAD_TRIM = True        # -1 trailing pads (descriptor trim)
SPOOL_BUFS = 4         # fp8 one-hot run buffers
GPOOL_BUFS = 12        # gather destination buffers
IPOOL_BUFS = 5         # index run buffers

# debug knobs (monkeypatched by bisect tests)
DBG_NLAYERS = 3
DBG_SKIP_GATHER = False
DBG_SKIP_COLLECTIVES = False
DBG_MAX_RUNS = None   # cap on gather runs per layer (bisect aid)
DBG_MAX_CALLS = None  # cap on gather calls per run (bisect aid)
DBG_NO_GATHER_CALLS = False  # keep matmuls, skip dma_gather instructions

LAST_RESULT = None


def kernel(**inputs):
    return _kernel(inputs, num_graphs=NUM_GRAPHS)


# ----------------------------------------------------------------------------
# Host-side structure + data preparation
# ----------------------------------------------------------------------------

def _prep(x, ei, batch, num_graphs):
    N, D = x.shape
    E = ei.shape[1]
    assert N % NCORES == 0
    SH = N // NCORES
    NB = -(-SH // BLK)
    SHP = NB * BLK
    NPASS = 4
    # quarter-window structure: window p = quarter p of every core's shard
    QB = [NB - 3 * (NB // 4)] + [NB // 4] * 3           # blocks per quarter
    QB = [25, 25, 24, 24] if NB == 98 else QB
    QSB = np.concatenate([[0], np.cumsum(QB)])           # block boundaries
    qrows = [q * BLK for q in QB]                        # rows per quarter
    qsr = (QSB[:4] * BLK).astype(np.int64)               # row starts
    for p in range(NPASS):
        assert NCORES * qrows[p] < 32768

    src = np.asarray(ei[0], dtype=np.int64)
    dst = np.asarray(ei[1], dtype=np.int64)
    batch = np.asarray(batch, dtype=np.int64)

    # pieces: phase A / AllGather emission granularity.  One piece per
    # quarter (small collectives have too much CC overhead), except the last
    # quarter is split at the final block group so only a tiny AllGather
    # remains exposed at the layer boundary.  Each quarter's window is laid
    # out [piece][core][piece rows] so a piece's AllGather is contiguous.
    NGROUPS = -(-NB // GSIZE)
    piece_spans = []
    for q in range(NPASS):
        lo, hi = int(QSB[q]), int(QSB[q + 1])
        split = (NGROUPS - 1) * GSIZE
        if q == NPASS - 1 and lo < split < hi:
            piece_spans += [(q, lo, split), (q, split, hi)]
        else:
            piece_spans.append((q, lo, hi))
    pieces = []
    piece_of_block = np.full(NB, -1, np.int64)
    hcum = [0] * NPASS
    for (q, blo, bhi) in piece_spans:
        prows = (bhi - blo) * BLK
        g = (bhi - 1) // GSIZE    # group whose completion finalizes the piece
        pieces.append(dict(q=q, g=g, blo=blo, bhi=bhi, prows=prows,
                           hofs=hcum[q], wofs=NCORES * hcum[q]))
        piece_of_block[blo:bhi] = len(pieces) - 1
        hcum[q] += prows
    assert hcum == qrows and (piece_of_block >= 0).all()
    blk_wofs = np.array([pieces[piece_of_block[b]]["wofs"]
                         for b in range(NB)], np.int64)
    blk_prows = np.array([pieces[piece_of_block[b]]["prows"]
                          for b in range(NB)], np.int64)
    blk_b0 = np.array([pieces[piece_of_block[b]]["blo"]
                       for b in range(NB)], np.int64)

    c_src = src // SH
    o_src = src % SH
    ob_src = o_src // BLK
    p_e = np.searchsorted(np.asarray(qsr[1:]), o_src, side="right")
    idx_e = (blk_wofs[ob_src] + c_src * blk_prows[ob_src]
             + (o_src - blk_b0[ob_src] * BLK)).astype(np.int16)
    c_e = dst // SH
    b_e = (dst % SH) // BLK
    off_e = (dst % SH) % BLK

    # bucket counts (bucket = (pass, block)), structure common to all cores
    cnt = np.zeros((NCORES, NPASS, NB), np.int64)
    np.add.at(cnt, (c_e, p_e, b_e), 1)
    maxcnt = cnt.max(axis=0)                             # [NPASS, NB]

    # layout: for bg (groups of GSIZE blocks): for p: a run whose buckets are
    # packed BACK-TO-BACK (slots per bucket = max-over-cores count, no
    # per-bucket round-up to 128); only the run total is tile-aligned.  A
    # 128-slot tile overlapping k buckets is matmul'd k times, each with its
    # own one-hot column group (zero rows mask out the other buckets' slots).
    # Calls are filled to GTILES_CAP tiles and may split buckets.
    calls = []       # dicts: p, tg0, ntile
    runs = []        # dicts: p, bg, tg0, mm0, ntiles, nmms, calls
    tile_call = []   # per tile: (call idx, tloc within call)
    tile_mms = []    # per tile: list of [mm, block, start, stop]
    mm_seq = []      # per matmul: [tile, block]
    bucket_s0 = np.full((NPASS, NB), -1, np.int64)  # global slot of bucket
    tg = 0
    mm = 0
    for bg in range(NGROUPS):
        blocks = list(range(bg * GSIZE, min((bg + 1) * GSIZE, NB)))
        for p in range(NPASS):
            run_buckets = [b for b in blocks if maxcnt[p, b] > 0]
            if not run_buckets:
                continue
            run = dict(p=p, bg=bg, tg0=tg, mm0=mm, ntiles=0, nmms=0, calls=[])
            spans = []
            cum = 0
            for b in run_buckets:
                bucket_s0[p, b] = tg * 128 + cum
                spans.append((b, cum, cum + int(maxcnt[p, b])))
                cum += int(maxcnt[p, b])
            run_tiles = -(-cum // 128)
            cur = None
            for t in range(run_tiles):
                if cur is None or cur["ntile"] == GTILES_CAP:
                    cur = dict(p=p, tg0=tg, ntile=0)
                    calls.append(cur)
                    run["calls"].append(len(calls) - 1)
                tile_call.append((len(calls) - 1, cur["ntile"]))
                lo, hi = t * 128, (t + 1) * 128
                mms_here = []
                for (b, blo, bhi) in spans:
                    if blo < hi and bhi > lo:
                        mms_here.append([mm, b, False, False])
                        mm_seq.append([tg, b])
                        mm += 1
                tile_mms.append(mms_here)
                cur["ntile"] += 1
                run["ntiles"] += 1
                run["nmms"] += len(mms_here)
                tg += 1
            runs.append(run)
    NT = tg
    M_total = mm
    S_total = NT * 128
    first_mm_of_block = {}
    last_mm_of_block = {}
    for j, (t, b) in enumerate(mm_seq):
        if b not in first_mm_of_block:
            first_mm_of_block[b] = j
        last_mm_of_block[b] = j
    assert len(first_mm_of_block) == NB, "every block needs an epilogue"
    for mms in tile_mms:
        for rec in mms:
            rec[2] = (first_mm_of_block[rec[1]] == rec[0])
            rec[3] = (last_mm_of_block[rec[1]] == rec[0])
    RUN_MAX = max(r["ntiles"] for r in runs)
    RUN_MMAX = max(r["nmms"] for r in runs)

    # ---- slot assignment (per core): edges sorted by src within bucket ----
    order = np.lexsort((idx_e, b_e, p_e, c_e))
    ckey = (c_e * NPASS + p_e) * NB + b_e
    kcnt = np.bincount(ckey, minlength=NCORES * NPASS * NB)
    kstart = np.concatenate([[0], np.cumsum(kcnt)])[:-1]
    rank = np.empty(E, np.int64)
    rank[order] = np.arange(E) - kstart[ckey[order]]
    pos = bucket_s0[p_e, b_e] + rank
    assert (rank < maxcnt[p_e, b_e]).all()

    # pad slots gather a *spread* of window rows (idx=0 for all pads would
    # serialize tens of thousands of reads on one HBM row); rows are spread
    # within each pass's window so every pad idx stays in range.  Each call's
    # per-core trailing pads get idx=-1 so the SWDGE trims their descriptors
    # -- but only within the call's FINAL 128-chunk: the decode stage
    # reserves ring space for ceil(num_idxs/128) chunks from the static
    # register, and a whole trimmed chunk would leave stale descriptors in
    # the ring for the next call to execute (engine fault).
    wrows = np.array([NCORES * q for q in qrows], np.int64)
    slot_pass = np.zeros(S_total, np.int64)
    for r in runs:
        slot_pass[r["tg0"] * 128:(r["tg0"] + r["ntiles"]) * 128] = r["p"]
    spread = (np.arange(S_total, dtype=np.int64) * 37) % wrows[slot_pass]
    idx_arr = np.broadcast_to(spread.astype(np.int16),
                              (NCORES, S_total)).copy()
    idx_arr[c_e, pos] = idx_e
    if PAD_TRIM:
        occ = np.zeros((NCORES, S_total), bool)
        occ[c_e, pos] = True
        for call in calls:
            c0 = call["tg0"] * 128
            ntile = call["ntile"]
            n = ntile * 128
            oseg = occ[:, c0:c0 + n]
            has = oseg.any(axis=1)
            last_real = np.where(has, n - 1 - np.argmax(oseg[:, ::-1], axis=1),
                                 -1)
            trail = np.maximum(last_real + 1, (ntile - 1) * 128 + 1)
            cols = np.arange(n)[None, :]
            idx_arr[:, c0:c0 + n][cols >= trail[:, None]] = -1

    # fp8 one-hot tiles, one 128-col group per MATMUL: sgm[c, m, j*128+off]=1
    # iff slot (tile_of(j), m) is an edge of block_of(j) with dst offset off
    mm_keys = np.array([t * NB + b for (t, b) in mm_seq], np.int64)
    assert (np.diff(mm_keys) > 0).all()
    edge_key = (pos // 128) * NB + b_e
    mm_e = np.searchsorted(mm_keys, edge_key)
    assert (mm_keys[mm_e] == edge_key).all()
    sgm = np.zeros((NCORES, 128, M_total * 128), np.uint8)
    sgm[c_e, pos % 128, mm_e * 128 + off_e] = FP8_ONE

    idx_dev = idx_arr.reshape(NCORES, S_total // 16, 16).transpose(0, 2, 1)
    idx_dev = np.ascontiguousarray(np.tile(idx_dev, (1, 8, 1)))  # [c,128,S/16]

    # host-precomputed symmetric-norm factors (deg includes self-loop)
    deg = np.bincount(dst, minlength=N).astype(np.float64) + 1.0
    dinv_full = (deg ** -0.5).astype(np.float32)
    dinvt = np.zeros((NCORES, SHP), np.float32)
    for c in range(NCORES):
        dinvt[c, :SH] = dinv_full[c * SH:(c + 1) * SH]
    dinvt = np.ascontiguousarray(
        dinvt.reshape(NCORES, NB, BLK).transpose(0, 2, 1))      # [c,128,NB]

    # per-core x shard (zero-padded, bf16) and fp8 graph-pool onehots
    import ml_dtypes
    xs = np.zeros((NCORES, SHP, D), ml_dtypes.bfloat16)
    xv = np.asarray(x, dtype=np.float32)
    GW = 512
    NGW = -(-num_graphs // GW)
    G_PAD = NGW * GW
    poolh = np.zeros((NCORES, 128, NB * NGW * GW), np.uint8)
    for c in range(NCORES):
        xs[c, :SH] = xv[c * SH:(c + 1) * SH].astype(ml_dtypes.bfloat16)
        bl = np.full(SHP, -1, np.int64)
        bl[:SH] = batch[c * SH:(c + 1) * SH]
        m = np.arange(SHP)
        valid = bl >= 0
        col = ((m // BLK) * NGW + bl // GW) * GW + bl % GW
        poolh[c, m[valid] % BLK, col[valid]] = FP8_ONE

    consts = np.eye(128, dtype=np.float32)

    struct = dict(
        N=N, D=D, E=E, SH=SH, NB=NB, SHP=SHP, NPASS=NPASS,
        NT=NT, M_total=M_total, S_total=S_total, calls=calls, runs=runs,
        tile_call=tile_call, tile_mms=tile_mms,
        RUN_MAX=RUN_MAX, RUN_MMAX=RUN_MMAX, QB=QB, QSB=QSB, qrows=qrows,
        pieces=pieces, NGROUPS=NGROUPS,
        G=num_graphs, GW=GW, NGW=NGW, G_PAD=G_PAD,
    )
    data = dict(xs=xs, idx=idx_dev, sgm=sgm, poolh=poolh, consts=consts,
                dinvt=dinvt)
    return struct, data


# ----------------------------------------------------------------------------
# Device program
# ----------------------------------------------------------------------------

def _build(st):
    import concourse.bacc as bacc
    import concourse.bass as bass  # noqa: F401
    import concourse.mybir as mybir
    import concourse.tile as tile

    f32 = mybir.dt.float32
    bf16 = mybir.dt.bfloat16
    fp8 = mybir.dt.float8e4
    i16 = mybir.dt.int16
    Alu = mybir.AluOpType
    Act = mybir.ActivationFunctionType

    D, H = st["D"], st["D"]
    NB, SHP, NPASS = st["NB"], st["SHP"], st["NPASS"]
    NT, S_total = st["NT"], st["S_total"]
    M_total = st["M_total"]
    pieces, NGROUPS = st["pieces"], st["NGROUPS"]
    RUN_MAX, RUN_MMAX = st["RUN_MAX"], st["RUN_MMAX"]
    QB, QSB, qrows = st["QB"], st["QSB"], st["qrows"]
    G = st["G"]
    GW, NGW, G_PAD = st["GW"], st["NGW"], st["G_PAD"]
    NGB = -(-G // 128)            # classifier graph blocks
    C = 10
    HC = 64                       # classifier hidden
    BNC = 1.0 / math.sqrt(1.0 + EPS)

    nc = bacc.Bacc("TRN2", target_bir_lowering=False, debug=False,
                   num_devices=NCORES,
                   dynamic_dma_scratch_size=DMA_SCRATCH,
                   num_swdge_queues=NQUEUES)

    xs_d = nc.dram_tensor("xs", [SHP, D], bf16, kind="ExternalInput")
    w_d = [nc.dram_tensor(f"w{l}", [D, H], f32, kind="ExternalInput")
           for l in range(3)]
    wc1_d = nc.dram_tensor("wc1", [H, HC], f32, kind="ExternalInput")
    wc2_d = nc.dram_tensor("wc2", [HC, C], f32, kind="ExternalInput")
    rows_d = nc.dram_tensor("rows", [1, 12 * 128], f32, kind="ExternalInput")
    idx_d = nc.dram_tensor("idx", [128, S_total // 16], i16, kind="ExternalInput")
    sgm_d = nc.dram_tensor("sgm", [128, M_total * 128], fp8,
                           kind="ExternalInput")
    poolh_d = nc.dram_tensor("poolh", [128, NB * NGW * GW], fp8,
                             kind="ExternalInput")
    dinvt_d = nc.dram_tensor("dinvt", [128, NB], f32, kind="ExternalInput")
    consts_d = nc.dram_tensor("consts", [128, 128], f32, kind="ExternalInput")
    out_d = nc.dram_tensor("out", [G, C], f32, kind="ExternalOutput")

    # double-buffered windows: layer l uses win[l % 2] so layer l+1's
    # AllGathers can run while layer l's gathers still read theirs.
    hq = [[nc.dram_tensor(f"hq{v}_{p}", [qrows[p], H], bf16)
           for p in range(NPASS)] for v in range(2)]
    win = [[nc.dram_tensor(f"win{v}_{p}", [NCORES * qrows[p], H], bf16,
                           addr_space="Shared") for p in range(NPASS)]
           for v in range(2)]
    pool_in = nc.dram_tensor("pool_in", [H, G_PAD], f32)
    pool_out = nc.dram_tensor("pool_out", [H, G_PAD], f32, addr_space="Shared")
    cnt_in = nc.dram_tensor("cnt_in", [1, G_PAD], f32)
    cnt_out = nc.dram_tensor("cnt_out", [1, G_PAD], f32, addr_space="Shared")

    calls, runs = st["calls"], st["runs"]
    tile_call, tile_mms = st["tile_call"], st["tile_mms"]
    call_tiles = [[] for _ in calls]   # per call: list of (tg, tloc)
    for tg, (ci, tloc) in enumerate(tile_call):
        call_tiles[ci].append((tg, tloc))

    with tile.TileContext(nc) as tc, ExitStack() as ctx:
        const = ctx.enter_context(tc.tile_pool(name="const", bufs=1))
        big = ctx.enter_context(tc.tile_pool(name="big", bufs=1))
        work = ctx.enter_context(tc.tile_pool(name="work", bufs=2))
        spool = ctx.enter_context(tc.tile_pool(name="spool", bufs=SPOOL_BUFS))
        gpool = ctx.enter_context(tc.tile_pool(name="gpool", bufs=GPOOL_BUFS))
        ipool = ctx.enter_context(tc.tile_pool(name="ipool", bufs=IPOOL_BUFS))
        ppool = ctx.enter_context(tc.tile_pool(name="ppool", bufs=3))

        # ------------- constants / persistent tiles -------------
        X = big.tile([128, NB * 128], bf16, tag="X")      # node features
        Y = big.tile([128, NB * 128], bf16, tag="Y")      # hhat (scaled h@W)
        ident = const.tile([128, 128], f32, tag="ident")
        ident_b = const.tile([128, 128], bf16, tag="ident_b")
        ones_col_b = const.tile([128, 1], bf16, tag="ones_col_b")
        ones_row = const.tile([1, 128], f32, tag="ones_row")
        dinv = const.tile([128, NB], f32, tag="dinv")
        rows_sb = const.tile([1, 12 * 128], f32, tag="rows")
        wc1_sb = const.tile([H, HC], f32, tag="wc1")
        wc2_sb = const.tile([HC, C], f32, tag="wc2")

        nc.vector.memset(ones_col_b[:], 1.0)
        nc.vector.memset(ones_row[:], 1.0)
        nc.sync.dma_start(ident[:], consts_d[:])
        nc.vector.tensor_copy(ident_b[:], ident[:])
        nc.sync.dma_start(rows_sb[:], rows_d[:])
        nc.sync.dma_start(wc1_sb[:], wc1_d[:])
        nc.sync.dma_start(wc2_sb[:], wc2_d[:])
        nc.sync.dma_start(dinv[:], dinvt_d[:])
        # x shard -> X  ([(b p), f] dram -> [p, (b, f)] sbuf)
        nc.sync.dma_start(
            X[:].rearrange("p (b f) -> p b f", b=NB),
            xs_d[:].rearrange("(b p) f -> p b f", p=128))

        # zero-init gather buffers once (descriptor-trimmed tail slots are
        # read by matmuls before any gather has written them)
        for _ in range(GPOOL_BUFS):
            gz = gpool.tile([128, GTILES_CAP, 128], bf16, tag="g")
            nc.vector.memset(gz[:], 0.0)

        # one register per distinct gather slot count
        nslot_reg = {}
        for call in calls:
            ns = call["ntile"] * 128
            # m2s/s2m descs per call = ns/16+1; ring holds DMA_SCRATCH/16
            assert ns // 16 + 1 <= DMA_SCRATCH // 16
            if ns not in nslot_reg:
                nslot_reg[ns] = nc.gpsimd.to_reg(ns)

        # ------------- layers -------------
        with (
            tc.tile_pool(name="psA", bufs=1, space="PSUM") as psA,
            tc.tile_pool(name="psS", bufs=GSIZE, space="PSUM") as psS,
            tc.tile_pool(name="wpool", bufs=4) as wpool,
        ):
            def wprep(layer):
                # wt = W * (g*BNC) per column; d_rep = (g*BNC*b + beta)
                # replicated across partitions
                wt = wpool.tile([D, H], bf16, tag="wt")
                drow = work.tile([1, 128], f32, tag="drow")
                d_rep = wpool.tile([128, 128], f32, tag="d_rep")
                grow = rows_sb[0:1, (3 * layer + 1) * 128:(3 * layer + 2) * 128]
                brow = rows_sb[0:1, (3 * layer + 0) * 128:(3 * layer + 1) * 128]
                berow = rows_sb[0:1, (3 * layer + 2) * 128:(3 * layer + 3) * 128]
                arep = psA.tile([128, 128], f32, tag="h1")
                nc.tensor.matmul(arep[:], ones_row[:], grow,
                                 start=True, stop=True)
                wsrc = work.tile([D, H], f32, tag="wsrc")
                nc.sync.dma_start(wsrc[:], w_d[layer][:])
                nc.vector.scalar_tensor_tensor(
                    wt[:], wsrc[:], BNC, arep[:], Alu.mult, Alu.mult)
                nc.vector.scalar_tensor_tensor(
                    drow[:], grow, BNC, brow, Alu.mult, Alu.mult)
                nc.vector.tensor_tensor(drow[:], drow[:], berow, Alu.add)
                drep_ps = psA.tile([128, 128], f32, tag="h1")
                nc.tensor.matmul(drep_ps[:], ones_row[:], drow[:],
                                 start=True, stop=True)
                nc.scalar.copy(d_rep[:], drep_ps[:])
                return wt, d_rep

            def phase_a_piece(pc, v, wt):
                # Y = dinv * (X @ wt) for the piece's blocks; DMA its rows
                # into the piece's hq slice (no collective)
                q, blo, bhi = pc["q"], pc["blo"], pc["bhi"]
                for b in range(blo, bhi):
                    xb = X[:, b * 128:(b + 1) * 128]
                    tp = psA.tile([128, 128], bf16, tag="tp")
                    nc.tensor.transpose(tp[:], xb, ident_b[:])
                    xT = work.tile([128, 128], bf16, tag="xT")
                    nc.scalar.copy(xT[:], tp[:])
                    h1 = psA.tile([128, 128], f32, tag="h1")
                    nc.tensor.matmul(h1[:], xT[:], wt[:],
                                     start=True, stop=True)
                    nc.scalar.mul(Y[:, b * 128:(b + 1) * 128], h1[:],
                                  dinv[:, b:b + 1])
                nc.sync.dma_start(
                    hq[v][q][pc["hofs"]:pc["hofs"] + pc["prows"]]
                    .rearrange("(b p) f -> p b f", p=128),
                    Y[:, blo * 128:bhi * 128]
                    .rearrange("p (b f) -> p b f", b=bhi - blo))

            def allgather_piece(pc, v):
                if not DBG_SKIP_COLLECTIVES:
                    q = pc["q"]
                    nc.gpsimd.collective_compute(
                        "AllGather", Alu.bypass,
                        replica_groups=[list(range(NCORES))],
                        ins=[hq[v][q][pc["hofs"]:pc["hofs"] + pc["prows"]]
                             .opt()],
                        outs=[win[v][q][pc["wofs"]:pc["wofs"]
                                        + NCORES * pc["prows"]].opt()],
                    )

            pieces_of_group = [[] for _ in range(NGROUPS)]
            for pc in pieces:
                pieces_of_group[pc["g"]].append(pc)

            wt, d_rep = wprep(0)
            for pc in pieces:
                phase_a_piece(pc, 0, wt)
                allgather_piece(pc, 0)

            cc_carry = []   # (piece, vbuf) AllGathers deferred cross-layer
            for layer in range(DBG_NLAYERS):
                v = layer % 2
                if DBG_SKIP_GATHER:
                    for b in range(NB):
                        nc.scalar.activation(
                            X[:, b * 128:(b + 1) * 128],
                            Y[:, b * 128:(b + 1) * 128], Act.Relu)
                    if layer + 1 < DBG_NLAYERS:
                        wt, d_rep = wprep(layer + 1)
                        for pc in pieces:
                            phase_a_piece(pc, 1 - v, wt)
                            allgather_piece(pc, 1 - v)
                    continue

                # next layer's weights are prepped up front so phase A of
                # layer+1 can be emitted piece-by-piece as each block group
                # finishes (overlapping its AllGathers with this layer's
                # gathers)
                d_rep_cur = d_rep
                last = layer + 1 >= DBG_NLAYERS
                if not last:
                    wt, d_rep = wprep(layer + 1)
                cc_pending = []    # phase-A'd pieces awaiting AllGather

                # -- phase C: gather + one-hot segment matmuls
                def load_run_tiles(run):
                    rtg0, rnt = run["tg0"], run["ntiles"]
                    rmm0, rnm = run["mm0"], run["nmms"]
                    s_run = spool.tile([128, RUN_MMAX * 128], fp8, tag="s")
                    nc.scalar.dma_start(
                        s_run[:, :rnm * 128],
                        sgm_d[:, rmm0 * 128:(rmm0 + rnm) * 128])
                    it = ipool.tile([128, RUN_MAX * 8], i16, tag="i")
                    nc.scalar.dma_start(
                        it[:, :rnt * 8],
                        idx_d[:, rtg0 * 8:(rtg0 + rnt) * 8])
                    return s_run, it

                acc_of_block = {}
                runs_used = runs if DBG_MAX_RUNS is None else runs[:DBG_MAX_RUNS]
                # s/idx tiles are prefetched one run ahead so the first
                # gather + matmul of a run never wait on their DMA
                next_tiles = load_run_tiles(runs_used[0]) if runs_used else None
                for ri, run in enumerate(runs_used):
                    p, rtg0, rnt = run["p"], run["tg0"], run["ntiles"]
                    rmm0 = run["mm0"]
                    # cross-layer deferred AllGathers: emit after the first
                    # run (their hq inputs are flowing by then), or forcibly
                    # before a run that consumes that quarter's window
                    while cc_carry and (ri >= 1 or cc_carry[0][0]["q"] == p):
                        cpc, cv = cc_carry.pop(0)
                        allgather_piece(cpc, cv)
                    s_run, it = next_tiles
                    if ri + 1 < len(runs_used):
                        next_tiles = load_run_tiles(runs_used[ri + 1])
                    calls_used = (run["calls"] if DBG_MAX_CALLS is None
                                  else run["calls"][:DBG_MAX_CALLS])
                    for ci in calls_used:
                        call = calls[ci]
                        ctg0, ntile = call["tg0"], call["ntile"]
                        nslot = ntile * 128
                        coff = (ctg0 - rtg0) * 8
                        gt = gpool.tile([128, GTILES_CAP, 128], bf16, tag="g")
                        if not DBG_NO_GATHER_CALLS:
                            nc.gpsimd.dma_gather(
                                gt[:, :ntile, :],
                                win[v][p][:],
                                it[:, coff:coff + nslot // 16],
                                num_idxs=nslot, num_idxs_reg=nslot_reg[nslot],
                                elem_size=H, queue_num=ci % NQUEUES,
                                single_packet=bool(SINGLE_PACKET),
                            )
                        for (tg, tl) in call_tiles[ci]:
                            for (jm, b, start, stop) in tile_mms[tg]:
                                if start:
                                    acc_of_block[b] = psS.tile(
                                        [128, 128], f32, name="acc", tag="acc")
                                acc = acc_of_block[b]
                                jl = jm - rmm0
                                nc.tensor.matmul(
                                    acc[:],
                                    s_run[:, jl * 128:(jl + 1) * 128],
                                    gt[:, tl, :],
                                    start=start, stop=stop)
                                if not stop:
                                    continue
                                # t1 = hhat_b + acc; X_b = relu(t1*dinv+d_rep)
                                yb = Y[:, b * 128:(b + 1) * 128]
                                tsum = work.tile([128, 128], f32, tag="tsum")
                                nc.vector.tensor_tensor(tsum[:], yb, acc[:],
                                                        Alu.add)
                                nc.vector.scalar_tensor_tensor(
                                    tsum[:], tsum[:], dinv[:, b:b + 1],
                                    d_rep_cur[:], Alu.mult, Alu.add)
                                nc.scalar.activation(
                                    X[:, b * 128:(b + 1) * 128], tsum[:],
                                    Act.Relu)
                    # after the last run of a block group: emit layer+1
                    # phase A for the group's pieces (their blocks are final);
                    # stagger each piece's AllGather one group later so the
                    # hq DMA has completed by the time GpSimd reaches it
                    if not last and run["p"] == NPASS - 1:
                        if run["bg"] < NGROUPS - 1:
                            while cc_pending:
                                allgather_piece(cc_pending.pop(0), 1 - v)
                        for pc in pieces_of_group[run["bg"]]:
                            phase_a_piece(pc, 1 - v, wt)
                            cc_pending.append(pc)
                if not last:
                    # leftover AllGathers (the final group's pieces) are
                    # deferred into the next layer's gather stream instead of
                    # stalling the GpSimd engine at the layer boundary
                    cc_carry = [(pc, 1 - v) for pc in cc_pending]

        # ------------- mean pool + classifier -------------
        with tc.tile_pool(name="psP", bufs=1, space="PSUM") as psP:
            pool_ps = [psP.tile([128, GW], f32, name=f"poolw{w}",
                                tag=f"pool{w}") for w in range(NGW)]
            cnt_ps = psP.tile([1, GW * NGW], f32, tag="cnt")
            for b in range(NB):
                xb = X[:, b * 128:(b + 1) * 128]
                pw = ppool.tile([128, NGW * GW], fp8, tag="pw")
                nc.sync.dma_start(
                    pw[:], poolh_d[:, b * NGW * GW:(b + 1) * NGW * GW])
                for w in range(NGW):
                    nc.tensor.matmul(pool_ps[w][:], xb,
                                     pw[:, w * GW:(w + 1) * GW],
                                     start=(b == 0), stop=(b == NB - 1))
                    nc.tensor.matmul(cnt_ps[:, w * GW:(w + 1) * GW],
                                     ones_col_b[:], pw[:, w * GW:(w + 1) * GW],
                                     start=(b == 0), stop=(b == NB - 1))
            pooledT = big.tile([128, G_PAD], f32, tag="pooledT")
            cnt_row = big.tile([1, G_PAD], f32, tag="cnt_row")
            for w in range(NGW):
                nc.scalar.copy(pooledT[:, w * GW:(w + 1) * GW], pool_ps[w][:])
            nc.scalar.copy(cnt_row[:], cnt_ps[:])
            nc.sync.dma_start(pool_in[:], pooledT[:])
            nc.sync.dma_start(cnt_in[:], cnt_row[:])
            if not DBG_SKIP_COLLECTIVES:
                nc.gpsimd.collective_compute(
                    "AllReduce", mybir.AluOpType.add,
                    replica_groups=[list(range(NCORES))],
                    ins=[pool_in[:].opt()], outs=[pool_out[:].opt()])
                nc.gpsimd.collective_compute(
                    "AllReduce", mybir.AluOpType.add,
                    replica_groups=[list(range(NCORES))],
                    ins=[cnt_in[:].opt()], outs=[cnt_out[:].opt()])
                nc.sync.dma_start(pooledT[:], pool_out[:])
                nc.sync.dma_start(cnt_row[:], cnt_out[:])

            # counts transposed: cntT[g%128, g//128] (per classifier block)
            cntT = big.tile([128, NGB], f32, tag="cntT")
            for k in range(NGB):
                ct = psP.tile([128, 1], f32, tag="ct")
                nc.tensor.transpose(
                    ct[:], cnt_row[0:1, k * 128:(k + 1) * 128],
                    ones_row[0:1, 0:1])
                nc.scalar.copy(cntT[:, k:k + 1], ct[:])
            nc.vector.tensor_scalar(cntT[:], cntT[:], 1.0, None, Alu.max)
            rcntT = big.tile([128, NGB], f32, tag="rcntT")
            nc.vector.reciprocal(rcntT[:], cntT[:])

            zT = big.tile([HC, NGB * 128], f32, tag="zT")
            for k in range(NGB):
                zp = psP.tile([128, HC], f32, tag="z")
                nc.tensor.matmul(zp[:], pooledT[:, k * 128:(k + 1) * 128],
                                 wc1_sb[:], start=True, stop=False)
                nc.tensor.matmul(zp[:], cnt_row[0:1, k * 128:(k + 1) * 128],
                                 rows_sb[0:1, 9 * 128:9 * 128 + HC],
                                 start=False, stop=True)
                zs = work.tile([128, HC], f32, tag="zs")
                nc.scalar.activation(zs[:], zp[:], Act.Relu,
                                     scale=rcntT[:, k:k + 1])
                ztp = psP.tile([HC, 128], f32, tag="ztp")
                nc.tensor.transpose(ztp[:], zs[:], ident[:])
                nc.scalar.copy(zT[:, k * 128:(k + 1) * 128], ztp[:])
            for k in range(NGB):
                op = psP.tile([128, C], f32, tag="o")
                nc.tensor.matmul(op[:], zT[:, k * 128:(k + 1) * 128],
                                 wc2_sb[:], start=True, stop=False)
                nc.tensor.matmul(op[:], ones_row[:],
                                 rows_sb[0:1, 10 * 128:10 * 128 + C],
                                 start=False, stop=True)
                ot = work.tile([128, C], f32, tag="ot")
                nc.scalar.copy(ot[:], op[:])
                nr = min(128, G - k * 128)
                nc.sync.dma_start(out_d[k * 128:k * 128 + nr, :], ot[:nr, :])

    nc.compile()
    return nc


# ----------------------------------------------------------------------------
# Entry point
# ----------------------------------------------------------------------------

def _pack_rows(inputs):
    rows = np.zeros((12, 128), np.float32)
    for l in range(3):
        rows[3 * l + 0, :128] = np.asarray(inputs[f"b{l + 1}"], np.float32)
        rows[3 * l + 1, :128] = np.asarray(inputs[f"g{l + 1}"], np.float32)
        rows[3 * l + 2, :128] = np.asarray(inputs[f"be{l + 1}"], np.float32)
    rows[9, :64] = np.asarray(inputs["bc1"], np.float32)
    rows[10, :10] = np.asarray(inputs["bc2"], np.float32)
    return rows.reshape(1, 12 * 128)


def _kernel(inputs, num_graphs):
    import ml_dtypes
    from concourse.bass_utils import run_bass_kernel_spmd

    x = np.ascontiguousarray(np.asarray(inputs["x"], dtype=np.float32))
    ei = np.asarray(inputs["edge_index"])
    batch = np.asarray(inputs["batch"])
    st, data = _prep(x, ei, batch, num_graphs)
    nc = _build(st)

    rows = _pack_rows(inputs)

    shared = dict(
        w0=np.ascontiguousarray(np.asarray(inputs["W1"], np.float32)),
        w1=np.ascontiguousarray(np.asarray(inputs["W2"], np.float32)),
        w2=np.ascontiguousarray(np.asarray(inputs["W3"], np.float32)),
        wc1=np.ascontiguousarray(np.asarray(inputs["Wc1"], np.float32)),
        wc2=np.ascontiguousarray(np.asarray(inputs["Wc2"], np.float32)),
        rows=rows,
        consts=np.ascontiguousarray(data["consts"]),
    )
    in_maps = []
    for c in range(NCORES):
        m = dict(shared)
        m["xs"] = np.ascontiguousarray(data["xs"][c])
        m["idx"] = np.ascontiguousarray(data["idx"][c])
        m["sgm"] = np.ascontiguousarray(data["sgm"][c]).view(ml_dtypes.float8_e4m3)
        m["poolh"] = np.ascontiguousarray(data["poolh"][c]).view(ml_dtypes.float8_e4m3)
        m["dinvt"] = np.ascontiguousarray(data["dinvt"][c])
        in_maps.append(m)

    import os
    trace = bool(os.environ.get("GCN_TRACE"))
    res = run_bass_kernel_spmd(
        nc, in_maps, core_ids=list(range(NCORES)), trace=trace)
    global LAST_RESULT
    LAST_RESULT = res
    return res.results[0]["out"]



# revision 48
# speedup vs baseline: 1.0005x; 1.0005x over previous
"""GCN (3x GCNConv + BN + ReLU, mean-pool, 2-layer MLP) on 8 Trainium2 cores.

Strategy (dst-sharded message passing, V2):
  - Nodes are dst-sharded: core c owns nodes [c*SH, (c+1)*SH).
  - Symmetric norm factorizes: out[i] = dinv[i] * sum_e dinv[src]*h'[src]
    so rows are scaled once (hhat = dinv * (h @ W)); dinv is host-precomputed.
  - hhat is exchanged in 4 quarter-window AllGathers (window p = quarter p of
    every core's shard, < 32768 rows for int16 gather indices) so gathers for
    pass p overlap the collective for pass p+1.
  - Per layer: dma_gather pulls 256B message rows from the window in HBM for
    the edges whose dst is local; a one-hot matmul segment-sums them in PSUM.
    One-hot S tiles are HOST-PRECOMPUTED fp8 and streamed from HBM (keeps the
    Vector engine and the GpSimd SWDGE descriptor generator from fighting
    over their shared SBUF port).
  - Edges bucketed by (pass window, dst block of 128), tiles padded to 128
    with a structure common to all 8 cores (single SPMD NEFF); pad slots in a
    call's final bucket carry idx=-1 so the SWDGE trims their descriptors.
  - Mean-pool via host-precomputed fp8 graph-onehot matmuls, AllReduce, then
    the classifier MLP runs (redundantly) on every core.
"""

import math
from contextlib import ExitStack

import numpy as np

NCORES = 8
NUM_GRAPHS = 1000  # G for the graded problem (not derivable from input shapes)
EPS = 1e-5

BLK = 128          # dst nodes per block (= one-hot matmul output partitions)
GSIZE = 6          # dst blocks whose PSUM accumulators are live at once
GTILES_CAP = 8     # max tiles per dma_gather call (64-desc/engine packet ceiling)
FP8_ONE = 0x38     # float8_e4m3 encoding of 1.0

DMA_SCRATCH = 16384    # SWDGE descriptor carveout (bytes per partition)
NQUEUES = 4            # SWDGE queues to round-robin gather calls over
SINGLE_PACKET = True   # dma_gather packetization mode
PAD_TRIM = True        # -1 trailing pads (descriptor trim)
SPOOL_BUFS = 4         # fp8 one-hot run buffers
GPOOL_BUFS = 12        # gather destination buffers
IPOOL_BUFS = 5         # index run buffers

# debug knobs (monkeypatched by bisect tests)
DBG_NLAYERS = 3
DBG_SKIP_GATHER = False
DBG_SKIP_COLLECTIVES = False
DBG_MAX_RUNS = None   # cap on gather runs per layer (bisect aid)
DBG_MAX_CALLS = None  # cap on gather calls per run (bisect aid)
DBG_NO_GATHER_CALLS = False  # keep matmuls, skip dma_gather instructions

LAST_RESULT = None


def kernel(**inputs):
    return _kernel(inputs, num_graphs=NUM_GRAPHS)


# ----------------------------------------------------------------------------
# Host-side structure + data preparation
# ----------------------------------------------------------------------------

def _prep(x, ei, batch, num_graphs):
    N, D = x.shape
    E = ei.shape[1]
    assert N % NCORES == 0
    SH = N // NCORES
    NB = -(-SH // BLK)
    SHP = NB * BLK
    NPASS = 4
    # quarter-window structure: window p = quarter p of every core's shard
    QB = [NB - 3 * (NB // 4)] + [NB // 4] * 3           # blocks per quarter
    QB = [25, 25, 24, 24] if NB == 98 else QB
    QSB = np.concatenate([[0], np.cumsum(QB)])           # block boundaries
    qrows = [q * BLK for q in QB]                        # rows per quarter
    qsr = (QSB[:4] * BLK).astype(np.int64)               # row starts
    for p in range(NPASS):
        assert NCORES * qrows[p] < 32768

    src = np.asarray(ei[0], dtype=np.int64)
    dst = np.asarray(ei[1], dtype=np.int64)
    batch = np.asarray(batch, dtype=np.int64)

    # pieces: phase A / AllGather emission granularity.  One piece per
    # quarter (small collectives have too much CC overhead), except the last
    # quarter is split at the final block group so only a tiny AllGather
    # remains exposed at the layer boundary.  Each quarter's window is laid
    # out [piece][core][piece rows] so a piece's AllGather is contiguous.
    NGROUPS = -(-NB // GSIZE)
    piece_spans = []
    for q in range(NPASS):
        lo, hi = int(QSB[q]), int(QSB[q + 1])
        split = (NGROUPS - 1) * GSIZE
        if q == NPASS - 1 and lo < split < hi:
            piece_spans += [(q, lo, split), (q, split, hi)]
        else:
            piece_spans.append((q, lo, hi))
    pieces = []
    piece_of_block = np.full(NB, -1, np.int64)
    hcum = [0] * NPASS
    for (q, blo, bhi) in piece_spans:
        prows = (bhi - blo) * BLK
        g = (bhi - 1) // GSIZE    # group whose completion finalizes the piece
        pieces.append(dict(q=q, g=g, blo=blo, bhi=bhi, prows=prows,
                           hofs=hcum[q], wofs=NCORES * hcum[q]))
        piece_of_block[blo:bhi] = len(pieces) - 1
        hcum[q] += prows
    assert hcum == qrows and (piece_of_block >= 0).all()
    blk_wofs = np.array([pieces[piece_of_block[b]]["wofs"]
                         for b in range(NB)], np.int64)
    blk_prows = np.array([pieces[piece_of_block[b]]["prows"]
                          for b in range(NB)], np.int64)
    blk_b0 = np.array([pieces[piece_of_block[b]]["blo"]
                       for b in range(NB)], np.int64)

    c_src = src // SH
    o_src = src % SH
    ob_src = o_src // BLK
    p_e = np.searchsorted(np.asarray(qsr[1:]), o_src, side="right")
    idx_e = (blk_wofs[ob_src] + c_src * blk_prows[ob_src]
             + (o_src - blk_b0[ob_src] * BLK)).astype(np.int16)
    c_e = dst // SH
    b_e = (dst % SH) // BLK
    off_e = (dst % SH) % BLK

    # bucket counts (bucket = (pass, block)), structure common to all cores
    cnt = np.zeros((NCORES, NPASS, NB), np.int64)
    np.add.at(cnt, (c_e, p_e, b_e), 1)
    maxcnt = cnt.max(axis=0)                             # [NPASS, NB]

    # layout: for bg (groups of GSIZE blocks): for p: a run whose buckets are
    # packed BACK-TO-BACK (slots per bucket = max-over-cores count, no
    # per-bucket round-up to 128); only the run total is tile-aligned.  A
    # 128-slot tile overlapping k buckets is matmul'd k times, each with its
    # own one-hot column group (zero rows mask out the other buckets' slots).
    # Calls are filled to GTILES_CAP tiles and may split buckets.
    calls = []       # dicts: p, tg0, ntile
    runs = []        # dicts: p, bg, tg0, mm0, ntiles, nmms, calls
    tile_call = []   # per tile: (call idx, tloc within call)
    tile_mms = []    # per tile: list of [mm, block, start, stop]
    mm_seq = []      # per matmul: [tile, block]
    bucket_s0 = np.full((NPASS, NB), -1, np.int64)  # global slot of bucket
    tg = 0
    mm = 0
    for bg in range(NGROUPS):
        blocks = list(range(bg * GSIZE, min((bg + 1) * GSIZE, NB)))
        for p in range(NPASS):
            run_buckets = [b for b in blocks if maxcnt[p, b] > 0]
            if not run_buckets:
                continue
            run = dict(p=p, bg=bg, tg0=tg, mm0=mm, ntiles=0, nmms=0, calls=[])
            spans = []
            cum = 0
            for b in run_buckets:
                bucket_s0[p, b] = tg * 128 + cum
                spans.append((b, cum, cum + int(maxcnt[p, b])))
                cum += int(maxcnt[p, b])
            run_tiles = -(-cum // 128)
            cur = None
            for t in range(run_tiles):
                if cur is None or cur["ntile"] == GTILES_CAP:
                    cur = dict(p=p, tg0=tg, ntile=0)
                    calls.append(cur)
                    run["calls"].append(len(calls) - 1)
                tile_call.append((len(calls) - 1, cur["ntile"]))
                lo, hi = t * 128, (t + 1) * 128
                mms_here = []
                for (b, blo, bhi) in spans:
                    if blo < hi and bhi > lo:
                        mms_here.append([mm, b, False, False])
                        mm_seq.append([tg, b])
                        mm += 1
                tile_mms.append(mms_here)
                cur["ntile"] += 1
                run["ntiles"] += 1
                run["nmms"] += len(mms_here)
                tg += 1
            runs.append(run)
    NT = tg
    M_total = mm
    S_total = NT * 128
    first_mm_of_block = {}
    last_mm_of_block = {}
    for j, (t, b) in enumerate(mm_seq):
        if b not in first_mm_of_block:
            first_mm_of_block[b] = j
        last_mm_of_block[b] = j
    assert len(first_mm_of_block) == NB, "every block needs an epilogue"
    for mms in tile_mms:
        for rec in mms:
            rec[2] = (first_mm_of_block[rec[1]] == rec[0])
            rec[3] = (last_mm_of_block[rec[1]] == rec[0])
    RUN_MAX = max(r["ntiles"] for r in runs)
    RUN_MMAX = max(r["nmms"] for r in runs)

    # ---- slot assignment (per core): edges sorted by src within bucket ----
    order = np.lexsort((idx_e, b_e, p_e, c_e))
    ckey = (c_e * NPASS + p_e) * NB + b_e
    kcnt = np.bincount(ckey, minlength=NCORES * NPASS * NB)
    kstart = np.concatenate([[0], np.cumsum(kcnt)])[:-1]
    rank = np.empty(E, np.int64)
    rank[order] = np.arange(E) - kstart[ckey[order]]
    pos = bucket_s0[p_e, b_e] + rank
    assert (rank < maxcnt[p_e, b_e]).all()

    # pad slots gather a *spread* of window rows (idx=0 for all pads would
    # serialize tens of thousands of reads on one HBM row); rows are spread
    # within each pass's window so every pad idx stays in range.  Each call's
    # per-core trailing pads get idx=-1 so the SWDGE trims their descriptors
    # -- but only within the call's FINAL 128-chunk: the decode stage
    # reserves ring space for ceil(num_idxs/128) chunks from the static
    # register, and a whole trimmed chunk would leave stale descriptors in
    # the ring for the next call to execute (engine fault).
    wrows = np.array([NCORES * q for q in qrows], np.int64)
    slot_pass = np.zeros(S_total, np.int64)
    for r in runs:
        slot_pass[r["tg0"] * 128:(r["tg0"] + r["ntiles"]) * 128] = r["p"]
    spread = (np.arange(S_total, dtype=np.int64) * 37) % wrows[slot_pass]
    idx_arr = np.broadcast_to(spread.astype(np.int16),
                              (NCORES, S_total)).copy()
    idx_arr[c_e, pos] = idx_e
    if PAD_TRIM:
        occ = np.zeros((NCORES, S_total), bool)
        occ[c_e, pos] = True
        for call in calls:
            c0 = call["tg0"] * 128
            ntile = call["ntile"]
            n = ntile * 128
            oseg = occ[:, c0:c0 + n]
            has = oseg.any(axis=1)
            last_real = np.where(has, n - 1 - np.argmax(oseg[:, ::-1], axis=1),
                                 -1)
            trail = np.maximum(last_real + 1, (ntile - 1) * 128 + 1)
            cols = np.arange(n)[None, :]
            idx_arr[:, c0:c0 + n][cols >= trail[:, None]] = -1

    # fp8 one-hot tiles, one 128-col group per MATMUL: sgm[c, m, j*128+off]=1
    # iff slot (tile_of(j), m) is an edge of block_of(j) with dst offset off
    mm_keys = np.array([t * NB + b for (t, b) in mm_seq], np.int64)
    assert (np.diff(mm_keys) > 0).all()
    edge_key = (pos // 128) * NB + b_e
    mm_e = np.searchsorted(mm_keys, edge_key)
    assert (mm_keys[mm_e] == edge_key).all()
    sgm = np.zeros((NCORES, 128, M_total * 128), np.uint8)
    sgm[c_e, pos % 128, mm_e * 128 + off_e] = FP8_ONE

    idx_dev = idx_arr.reshape(NCORES, S_total // 16, 16).transpose(0, 2, 1)
    idx_dev = np.ascontiguousarray(np.tile(idx_dev, (1, 8, 1)))  # [c,128,S/16]

    # host-precomputed symmetric-norm factors (deg includes self-loop)
    deg = np.bincount(dst, minlength=N).astype(np.float64) + 1.0
    dinv_full = (deg ** -0.5).astype(np.float32)
    dinvt = np.zeros((NCORES, SHP), np.float32)
    for c in range(NCORES):
        dinvt[c, :SH] = dinv_full[c * SH:(c + 1) * SH]
    dinvt = np.ascontiguousarray(
        dinvt.reshape(NCORES, NB, BLK).transpose(0, 2, 1))      # [c,128,NB]

    # per-core x shard (zero-padded, bf16) and fp8 graph-pool onehots
    import ml_dtypes
    xs = np.zeros((NCORES, SHP, D), ml_dtypes.bfloat16)
    xv = np.asarray(x, dtype=np.float32)
    GW = 512
    NGW = -(-num_graphs // GW)
    G_PAD = NGW * GW
    poolh = np.zeros((NCORES, 128, NB * NGW * GW), np.uint8)
    for c in range(NCORES):
        xs[c, :SH] = xv[c * SH:(c + 1) * SH].astype(ml_dtypes.bfloat16)
        bl = np.full(SHP, -1, np.int64)
        bl[:SH] = batch[c * SH:(c + 1) * SH]
        m = np.arange(SHP)
        valid = bl >= 0
        col = ((m // BLK) * NGW + bl // GW) * GW + bl % GW
        poolh[c, m[valid] % BLK, col[valid]] = FP8_ONE

    consts = np.eye(128, dtype=np.float32)

    struct = dict(
        N=N, D=D, E=E, SH=SH, NB=NB, SHP=SHP, NPASS=NPASS,
        NT=NT, M_total=M_total, S_total=S_total, calls=calls, runs=runs,
        tile_call=tile_call, tile_mms=tile_mms,
        RUN_MAX=RUN_MAX, RUN_MMAX=RUN_MMAX, QB=QB, QSB=QSB, qrows=qrows,
        pieces=pieces, NGROUPS=NGROUPS,
        G=num_graphs, GW=GW, NGW=NGW, G_PAD=G_PAD,
    )
    data = dict(xs=xs, idx=idx_dev, sgm=sgm, poolh=poolh, consts=consts,
                dinvt=dinvt)
    return struct, data


# ----------------------------------------------------------------------------
# Device program
# ----------------------------------------------------------------------------

def _build(st):
    import concourse.bacc as bacc
    import concourse.bass as bass  # noqa: F401
    import concourse.mybir as mybir
    import concourse.tile as tile

    f32 = mybir.dt.float32
    bf16 = mybir.dt.bfloat16
    fp8 = mybir.dt.float8e4
    i16 = mybir.dt.int16
    Alu = mybir.AluOpType
    Act = mybir.ActivationFunctionType

    D, H = st["D"], st["D"]
    NB, SHP, NPASS = st["NB"], st["SHP"], st["NPASS"]
    NT, S_total = st["NT"], st["S_total"]
    M_total = st["M_total"]
    pieces, NGROUPS = st["pieces"], st["NGROUPS"]
    RUN_MAX, RUN_MMAX = st["RUN_MAX"], st["RUN_MMAX"]
    QB, QSB, qrows = st["QB"], st["QSB"], st["qrows"]
    G = st["G"]
    GW, NGW, G_PAD = st["GW"], st["NGW"], st["G_PAD"]
    NGB = -(-G // 128)            # classifier graph blocks
    C = 10
    HC = 64                       # classifier hidden
    BNC = 1.0 / math.sqrt(1.0 + EPS)

    nc = bacc.Bacc("TRN2", target_bir_lowering=False, debug=False,
                   num_devices=NCORES,
                   dynamic_dma_scratch_size=DMA_SCRATCH,
                   num_swdge_queues=NQUEUES)

    xs_d = nc.dram_tensor("xs", [SHP, D], bf16, kind="ExternalInput")
    w_d = [nc.dram_tensor(f"w{l}", [D, H], f32, kind="ExternalInput")
           for l in range(3)]
    wc1_d = nc.dram_tensor("wc1", [H, HC], f32, kind="ExternalInput")
    wc2_d = nc.dram_tensor("wc2", [HC, C], f32, kind="ExternalInput")
    rows_d = nc.dram_tensor("rows", [1, 12 * 128], f32, kind="ExternalInput")
    idx_d = nc.dram_tensor("idx", [128, S_total // 16], i16, kind="ExternalInput")
    sgm_d = nc.dram_tensor("sgm", [128, M_total * 128], fp8,
                           kind="ExternalInput")
    poolh_d = nc.dram_tensor("poolh", [128, NB * NGW * GW], fp8,
                             kind="ExternalInput")
    dinvt_d = nc.dram_tensor("dinvt", [128, NB], f32, kind="ExternalInput")
    consts_d = nc.dram_tensor("consts", [128, 128], f32, kind="ExternalInput")
    out_d = nc.dram_tensor("out", [G, C], f32, kind="ExternalOutput")

    # double-buffered windows: layer l uses win[l % 2] so layer l+1's
    # AllGathers can run while layer l's gathers still read theirs.
    hq = [[nc.dram_tensor(f"hq{v}_{p}", [qrows[p], H], bf16)
           for p in range(NPASS)] for v in range(2)]
    win = [[nc.dram_tensor(f"win{v}_{p}", [NCORES * qrows[p], H], bf16,
                           addr_space="Shared") for p in range(NPASS)]
           for v in range(2)]
    pool_in = nc.dram_tensor("pool_in", [H, G_PAD], f32)
    pool_out = nc.dram_tensor("pool_out", [H, G_PAD], f32, addr_space="Shared")
    cnt_in = nc.dram_tensor("cnt_in", [1, G_PAD], f32)
    cnt_out = nc.dram_tensor("cnt_out", [1, G_PAD], f32, addr_space="Shared")

    calls, runs = st["calls"], st["runs"]
    tile_call, tile_mms = st["tile_call"], st["tile_mms"]
    call_tiles = [[] for _ in calls]   # per call: list of (tg, tloc)
    for tg, (ci, tloc) in enumerate(tile_call):
        call_tiles[ci].append((tg, tloc))

    with tile.TileContext(nc) as tc, ExitStack() as ctx:
        const = ctx.enter_context(tc.tile_pool(name="const", bufs=1))
        big = ctx.enter_context(tc.tile_pool(name="big", bufs=1))
        work = ctx.enter_context(tc.tile_pool(name="work", bufs=2))
        spool = ctx.enter_context(tc.tile_pool(name="spool", bufs=SPOOL_BUFS))
        gpool = ctx.enter_context(tc.tile_pool(name="gpool", bufs=GPOOL_BUFS))
        ipool = ctx.enter_context(tc.tile_pool(name="ipool", bufs=IPOOL_BUFS))
        ppool = ctx.enter_context(tc.tile_pool(name="ppool", bufs=3))

        # ------------- constants / persistent tiles -------------
        X = big.tile([128, NB * 128], bf16, tag="X")      # node features
        Y = big.tile([128, NB * 128], bf16, tag="Y")      # hhat (scaled h@W)
        ident = const.tile([128, 128], f32, tag="ident")
        ident_b = const.tile([128, 128], bf16, tag="ident_b")
        ones_col_b = const.tile([128, 1], bf16, tag="ones_col_b")
        ones_row = const.tile([1, 128], f32, tag="ones_row")
        dinv = const.tile([128, NB], f32, tag="dinv")
        rows_sb = const.tile([1, 12 * 128], f32, tag="rows")
        wc1_sb = const.tile([H, HC], f32, tag="wc1")
        wc2_sb = const.tile([HC, C], f32, tag="wc2")

        nc.vector.memset(ones_col_b[:], 1.0)
        nc.vector.memset(ones_row[:], 1.0)
        nc.sync.dma_start(ident[:], consts_d[:])
        nc.vector.tensor_copy(ident_b[:], ident[:])
        nc.sync.dma_start(rows_sb[:], rows_d[:])
        nc.sync.dma_start(wc1_sb[:], wc1_d[:])
        nc.sync.dma_start(wc2_sb[:], wc2_d[:])
        nc.sync.dma_start(dinv[:], dinvt_d[:])
        # x shard -> X  ([(b p), f] dram -> [p, (b, f)] sbuf)
        nc.sync.dma_start(
            X[:].rearrange("p (b f) -> p b f", b=NB),
            xs_d[:].rearrange("(b p) f -> p b f", p=128))

        # zero-init gather buffers once (descriptor-trimmed tail slots are
        # read by matmuls before any gather has written them)
        for _ in range(GPOOL_BUFS):
            gz = gpool.tile([128, GTILES_CAP, 128], bf16, tag="g")
            nc.vector.memset(gz[:], 0.0)

        # one register per distinct gather slot count
        nslot_reg = {}
        for call in calls:
            ns = call["ntile"] * 128
            # m2s/s2m descs per call = ns/16+1; ring holds DMA_SCRATCH/16
            assert ns // 16 + 1 <= DMA_SCRATCH // 16
            if ns not in nslot_reg:
                nslot_reg[ns] = nc.gpsimd.to_reg(ns)

        # ------------- layers -------------
        with (
            tc.tile_pool(name="psA", bufs=1, space="PSUM") as psA,
            tc.tile_pool(name="psS", bufs=GSIZE, space="PSUM") as psS,
            tc.tile_pool(name="wpool", bufs=4) as wpool,
        ):
            def wprep(layer):
                # wt = W * (g*BNC) per column; d_rep = (g*BNC*b + beta)
                # replicated across partitions
                wt = wpool.tile([D, H], bf16, tag="wt")
                drow = work.tile([1, 128], f32, tag="drow")
                d_rep = wpool.tile([128, 128], f32, tag="d_rep")
                grow = rows_sb[0:1, (3 * layer + 1) * 128:(3 * layer + 2) * 128]
                brow = rows_sb[0:1, (3 * layer + 0) * 128:(3 * layer + 1) * 128]
                berow = rows_sb[0:1, (3 * layer + 2) * 128:(3 * layer + 3) * 128]
                arep = psA.tile([128, 128], f32, tag="h1")
                nc.tensor.matmul(arep[:], ones_row[:], grow,
                                 start=True, stop=True)
                wsrc = work.tile([D, H], f32, tag="wsrc")
                nc.sync.dma_start(wsrc[:], w_d[layer][:])
                nc.vector.scalar_tensor_tensor(
                    wt[:], wsrc[:], BNC, arep[:], Alu.mult, Alu.mult)
                nc.vector.scalar_tensor_tensor(
                    drow[:], grow, BNC, brow, Alu.mult, Alu.mult)
                nc.vector.tensor_tensor(drow[:], drow[:], berow, Alu.add)
                drep_ps = psA.tile([128, 128], f32, tag="h1")
                nc.tensor.matmul(drep_ps[:], ones_row[:], drow[:],
                                 start=True, stop=True)
                nc.scalar.copy(d_rep[:], drep_ps[:])
                return wt, d_rep

            def phase_a_piece(pc, v, wt):
                # Y = dinv * (X @ wt) for the piece's blocks; DMA its rows
                # into the piece's hq slice (no collective)
                q, blo, bhi = pc["q"], pc["blo"], pc["bhi"]
                for b in range(blo, bhi):
                    xb = X[:, b * 128:(b + 1) * 128]
                    tp = psA.tile([128, 128], bf16, tag="tp")
                    nc.tensor.transpose(tp[:], xb, ident_b[:])
                    xT = work.tile([128, 128], bf16, tag="xT")
                    nc.scalar.copy(xT[:], tp[:])
                    h1 = psA.tile([128, 128], f32, tag="h1")
                    nc.tensor.matmul(h1[:], xT[:], wt[:],
                                     start=True, stop=True)
                    nc.scalar.mul(Y[:, b * 128:(b + 1) * 128], h1[:],
                                  dinv[:, b:b + 1])
                nc.sync.dma_start(
                    hq[v][q][pc["hofs"]:pc["hofs"] + pc["prows"]]
                    .rearrange("(b p) f -> p b f", p=128),
                    Y[:, blo * 128:bhi * 128]
                    .rearrange("p (b f) -> p b f", b=bhi - blo))

            def allgather_piece(pc, v):
                if not DBG_SKIP_COLLECTIVES:
                    q = pc["q"]
                    nc.gpsimd.collective_compute(
                        "AllGather", Alu.bypass,
                        replica_groups=[list(range(NCORES))],
                        ins=[hq[v][q][pc["hofs"]:pc["hofs"] + pc["prows"]]
                             .opt()],
                        outs=[win[v][q][pc["wofs"]:pc["wofs"]
                                        + NCORES * pc["prows"]].opt()],
                    )

            pieces_of_group = [[] for _ in range(NGROUPS)]
            for pc in pieces:
                pieces_of_group[pc["g"]].append(pc)

            wt, d_rep = wprep(0)
            for pc in pieces:
                phase_a_piece(pc, 0, wt)

            # Collectives serialize: an AllGather instruction blocks the
            # GpSimd stream until the PREVIOUS collective completes
            # (straight-line collective ordering), so dispatches are spaced
            # >= CC_SPACING runs apart in the gather stream.  A run whose
            # pass needs a still-queued window AllGather force-drains it.
            CC_SPACING = 4
            cc_carry = [(pc, 0) for pc in pieces]
            for layer in range(DBG_NLAYERS):
                v = layer % 2
                if DBG_SKIP_GATHER:
                    for b in range(NB):
                        nc.scalar.activation(
                            X[:, b * 128:(b + 1) * 128],
                            Y[:, b * 128:(b + 1) * 128], Act.Relu)
                    if layer + 1 < DBG_NLAYERS:
                        wt, d_rep = wprep(layer + 1)
                        for pc in pieces:
                            phase_a_piece(pc, 1 - v, wt)
                            allgather_piece(pc, 1 - v)
                    continue

                # next layer's weights are prepped up front so phase A of
                # layer+1 can be emitted piece-by-piece as each block group
                # finishes (overlapping its AllGathers with this layer's
                # gathers)
                d_rep_cur = d_rep
                last = layer + 1 >= DBG_NLAYERS
                if not last:
                    wt, d_rep = wprep(layer + 1)
                runs_since_cc = CC_SPACING  # first run may dispatch one

                # -- phase C: gather + one-hot segment matmuls
                def load_run_tiles(run):
                    rtg0, rnt = run["tg0"], run["ntiles"]
                    rmm0, rnm = run["mm0"], run["nmms"]
                    s_run = spool.tile([128, RUN_MMAX * 128], fp8, tag="s")
                    nc.scalar.dma_start(
                        s_run[:, :rnm * 128],
                        sgm_d[:, rmm0 * 128:(rmm0 + rnm) * 128])
                    it = ipool.tile([128, RUN_MAX * 8], i16, tag="i")
                    nc.scalar.dma_start(
                        it[:, :rnt * 8],
                        idx_d[:, rtg0 * 8:(rtg0 + rnt) * 8])
                    return s_run, it

                acc_of_block = {}
                runs_used = runs if DBG_MAX_RUNS is None else runs[:DBG_MAX_RUNS]
                # s/idx tiles are prefetched one run ahead so the first
                # gather + matmul of a run never wait on their DMA
                next_tiles = load_run_tiles(runs_used[0]) if runs_used else None
                for ri, run in enumerate(runs_used):
                    p, rtg0, rnt = run["p"], run["tg0"], run["ntiles"]
                    rmm0 = run["mm0"]
                    # deferred AllGathers: force-drain everything this run's
                    # pass needs (current-layer windows, quarter <= p);
                    # otherwise dispatch at most one per CC_SPACING runs
                    runs_since_cc += 1
                    while cc_carry and cc_carry[0][1] == v \
                            and cc_carry[0][0]["q"] <= p:
                        allgather_piece(*cc_carry.pop(0))
                        runs_since_cc = 0
                    if cc_carry and runs_since_cc >= CC_SPACING:
                        allgather_piece(*cc_carry.pop(0))
                        runs_since_cc = 0
                    s_run, it = next_tiles
                    if ri + 1 < len(runs_used):
                        next_tiles = load_run_tiles(runs_used[ri + 1])
                    calls_used = (run["calls"] if DBG_MAX_CALLS is None
                                  else run["calls"][:DBG_MAX_CALLS])
                    for ci in calls_used:
                        call = calls[ci]
                        ctg0, ntile = call["tg0"], call["ntile"]
                        nslot = ntile * 128
                        coff = (ctg0 - rtg0) * 8
                        gt = gpool.tile([128, GTILES_CAP, 128], bf16, tag="g")
                        if not DBG_NO_GATHER_CALLS:
                            nc.gpsimd.dma_gather(
                                gt[:, :ntile, :],
                                win[v][p][:],
                                it[:, coff:coff + nslot // 16],
                                num_idxs=nslot, num_idxs_reg=nslot_reg[nslot],
                                elem_size=H, queue_num=ci % NQUEUES,
                                single_packet=bool(SINGLE_PACKET),
                            )
                        for (tg, tl) in call_tiles[ci]:
                            for (jm, b, start, stop) in tile_mms[tg]:
                                if start:
                                    acc_of_block[b] = psS.tile(
                                        [128, 128], f32, name="acc", tag="acc")
                                acc = acc_of_block[b]
                                jl = jm - rmm0
                                nc.tensor.matmul(
                                    acc[:],
                                    s_run[:, jl * 128:(jl + 1) * 128],
                                    gt[:, tl, :],
                                    start=start, stop=stop)
                                if not stop:
                                    continue
                                # t1 = hhat_b + acc; X_b = relu(t1*dinv+d_rep)
                                yb = Y[:, b * 128:(b + 1) * 128]
                                tsum = work.tile([128, 128], f32, tag="tsum")
                                nc.vector.tensor_tensor(tsum[:], yb, acc[:],
                                                        Alu.add)
                                nc.vector.scalar_tensor_tensor(
                                    tsum[:], tsum[:], dinv[:, b:b + 1],
                                    d_rep_cur[:], Alu.mult, Alu.add)
                                nc.scalar.activation(
                                    X[:, b * 128:(b + 1) * 128], tsum[:],
                                    Act.Relu)
                    # after the last run of a block group: emit layer+1
                    # phase A for the group's pieces (their blocks are
                    # final); their AllGathers join the paced queue
                    if not last and run["p"] == NPASS - 1:
                        for pc in pieces_of_group[run["bg"]]:
                            phase_a_piece(pc, 1 - v, wt)
                            cc_carry.append((pc, 1 - v))

        # ------------- mean pool + classifier -------------
        with tc.tile_pool(name="psP", bufs=1, space="PSUM") as psP:
            pool_ps = [psP.tile([128, GW], f32, name=f"poolw{w}",
                                tag=f"pool{w}") for w in range(NGW)]
            cnt_ps = psP.tile([1, GW * NGW], f32, tag="cnt")
            for b in range(NB):
                xb = X[:, b * 128:(b + 1) * 128]
                pw = ppool.tile([128, NGW * GW], fp8, tag="pw")
                nc.sync.dma_start(
                    pw[:], poolh_d[:, b * NGW * GW:(b + 1) * NGW * GW])
                for w in range(NGW):
                    nc.tensor.matmul(pool_ps[w][:], xb,
                                     pw[:, w * GW:(w + 1) * GW],
                                     start=(b == 0), stop=(b == NB - 1))
                    nc.tensor.matmul(cnt_ps[:, w * GW:(w + 1) * GW],
                                     ones_col_b[:], pw[:, w * GW:(w + 1) * GW],
                                     start=(b == 0), stop=(b == NB - 1))
            pooledT = big.tile([128, G_PAD], f32, tag="pooledT")
            cnt_row = big.tile([1, G_PAD], f32, tag="cnt_row")
            for w in range(NGW):
                nc.scalar.copy(pooledT[:, w * GW:(w + 1) * GW], pool_ps[w][:])
            nc.scalar.copy(cnt_row[:], cnt_ps[:])
            nc.sync.dma_start(pool_in[:], pooledT[:])
            nc.sync.dma_start(cnt_in[:], cnt_row[:])
            if not DBG_SKIP_COLLECTIVES:
                nc.gpsimd.collective_compute(
                    "AllReduce", mybir.AluOpType.add,
                    replica_groups=[list(range(NCORES))],
                    ins=[pool_in[:].opt()], outs=[pool_out[:].opt()])
                nc.gpsimd.collective_compute(
                    "AllReduce", mybir.AluOpType.add,
                    replica_groups=[list(range(NCORES))],
                    ins=[cnt_in[:].opt()], outs=[cnt_out[:].opt()])
                nc.sync.dma_start(pooledT[:], pool_out[:])
                nc.sync.dma_start(cnt_row[:], cnt_out[:])

            # counts transposed: cntT[g%128, g//128] (per classifier block)
            cntT = big.tile([128, NGB], f32, tag="cntT")
            for k in range(NGB):
                ct = psP.tile([128, 1], f32, tag="ct")
                nc.tensor.transpose(
                    ct[:], cnt_row[0:1, k * 128:(k + 1) * 128],
                    ones_row[0:1, 0:1])
                nc.scalar.copy(cntT[:, k:k + 1], ct[:])
            nc.vector.tensor_scalar(cntT[:], cntT[:], 1.0, None, Alu.max)
            rcntT = big.tile([128, NGB], f32, tag="rcntT")
            nc.vector.reciprocal(rcntT[:], cntT[:])

            zT = big.tile([HC, NGB * 128], f32, tag="zT")
            for k in range(NGB):
                zp = psP.tile([128, HC], f32, tag="z")
                nc.tensor.matmul(zp[:], pooledT[:, k * 128:(k + 1) * 128],
                                 wc1_sb[:], start=True, stop=False)
                nc.tensor.matmul(zp[:], cnt_row[0:1, k * 128:(k + 1) * 128],
                                 rows_sb[0:1, 9 * 128:9 * 128 + HC],
                                 start=False, stop=True)
                zs = work.tile([128, HC], f32, tag="zs")
                nc.scalar.activation(zs[:], zp[:], Act.Relu,
                                     scale=rcntT[:, k:k + 1])
                ztp = psP.tile([HC, 128], f32, tag="ztp")
                nc.tensor.transpose(ztp[:], zs[:], ident[:])
                nc.scalar.copy(zT[:, k * 128:(k + 1) * 128], ztp[:])
            for k in range(NGB):
                op = psP.tile([128, C], f32, tag="o")
                nc.tensor.matmul(op[:], zT[:, k * 128:(k + 1) * 128],
                                 wc2_sb[:], start=True, stop=False)
                nc.tensor.matmul(op[:], ones_row[:],
                                 rows_sb[0:1, 10 * 128:10 * 128 + C],
                                 start=False, stop=True)
                ot = work.tile([128, C], f32, tag="ot")
                nc.scalar.copy(ot[:], op[:])
                nr = min(128, G - k * 128)
                nc.sync.dma_start(out_d[k * 128:k * 128 + nr, :], ot[:nr, :])

    nc.compile()
    return nc


# ----------------------------------------------------------------------------
# Entry point
# ----------------------------------------------------------------------------

def _pack_rows(inputs):
    rows = np.zeros((12, 128), np.float32)
    for l in range(3):
        rows[3 * l + 0, :128] = np.asarray(inputs[f"b{l + 1}"], np.float32)
        rows[3 * l + 1, :128] = np.asarray(inputs[f"g{l + 1}"], np.float32)
        rows[3 * l + 2, :128] = np.asarray(inputs[f"be{l + 1}"], np.float32)
    rows[9, :64] = np.asarray(inputs["bc1"], np.float32)
    rows[10, :10] = np.asarray(inputs["bc2"], np.float32)
    return rows.reshape(1, 12 * 128)


def _kernel(inputs, num_graphs):
    import ml_dtypes
    from concourse.bass_utils import run_bass_kernel_spmd

    x = np.ascontiguousarray(np.asarray(inputs["x"], dtype=np.float32))
    ei = np.asarray(inputs["edge_index"])
    batch = np.asarray(inputs["batch"])
    st, data = _prep(x, ei, batch, num_graphs)
    nc = _build(st)

    rows = _pack_rows(inputs)

    shared = dict(
        w0=np.ascontiguousarray(np.asarray(inputs["W1"], np.float32)),
        w1=np.ascontiguousarray(np.asarray(inputs["W2"], np.float32)),
        w2=np.ascontiguousarray(np.asarray(inputs["W3"], np.float32)),
        wc1=np.ascontiguousarray(np.asarray(inputs["Wc1"], np.float32)),
        wc2=np.ascontiguousarray(np.asarray(inputs["Wc2"], np.float32)),
        rows=rows,
        consts=np.ascontiguousarray(data["consts"]),
    )
    in_maps = []
    for c in range(NCORES):
        m = dict(shared)
        m["xs"] = np.ascontiguousarray(data["xs"][c])
        m["idx"] = np.ascontiguousarray(data["idx"][c])
        m["sgm"] = np.ascontiguousarray(data["sgm"][c]).view(ml_dtypes.float8_e4m3)
        m["poolh"] = np.ascontiguousarray(data["poolh"][c]).view(ml_dtypes.float8_e4m3)
        m["dinvt"] = np.ascontiguousarray(data["dinvt"][c])
        in_maps.append(m)

    import os
    trace = bool(os.environ.get("GCN_TRACE"))
    res = run_bass_kernel_spmd(
        nc, in_maps, core_ids=list(range(NCORES)), trace=trace)
    global LAST_RESULT
    LAST_RESULT = res
    return res.results[0]["out"]

